# revision 3
# baseline (speedup 1.0000x reference)
"""Causal self-attention kernel for 8 Trainium2 NeuronCores.

Sharding: batch (2) x head-groups (4 heads each) -> 8 cores.
Each core computes, for its (batch b, 4 heads):
  - QK^T in transposed layout (d, s) and V in (s, d) via one fused QKV
    projection pass (fp32r matmuls, bias folded in via a ones-row matmul)
  - per-head causal scores + softmax (no max subtraction needed: scores
    are ~N(0,1); exp on ACT with fused row-sum accumulation)
  - normalized attention weights (written to DRAM; upper triangle is
    skipped and comes from the zero-initialized output buffer)
  - attended = attn @ V via PE-transposed attention tiles
  - its partial output projection x W_out[head rows]; the host sums the
    4 per-core partials per batch and adds the output bias.

All matmuls use float32r (TF32-like, ~1e-4 rel err, full PE rate).
"""

import numpy as np
import concourse.bacc as bacc
import concourse.mybir as mybir
from concourse import tile
from concourse.bass_utils import run_bass_kernel_spmd
from concourse.masks import make_identity

B, S, E, H, D = 2, 2048, 1024, 16, 64
NCORES = 8
HPC = 4                      # heads per core
P = 128
NQT = S // P                 # 16 q-tiles
QC2 = 256                    # attended q-chunk width
f32 = mybir.dt.float32
f32r = mybir.dt.float32r
EXP = mybir.ActivationFunctionType.Exp
AX = mybir.AxisListType.X
IS_GE = mybir.AluOpType.is_ge

_prog_cache = {}


def _build(reps=1):
    nc = bacc.Bacc(None, target_bir_lowering=False)
    xT_d = nc.dram_tensor("xT", [E + 1, S], f32r, kind="ExternalInput")
    wqkv_d = nc.dram_tensor("wqkv", [E + 1, 3 * HPC * D], f32r, kind="ExternalInput")
    wout_d = nc.dram_tensor("wout", [HPC * D, E], f32r, kind="ExternalInput")
    attn_d = nc.dram_tensor("attn", [HPC, S, S], f32r, kind="ExternalOutput")
    outp_d = nc.dram_tensor("outp", [S, E], f32, kind="ExternalOutput")

    with tile.TileContext(nc) as tc:
        with tc.tile_pool(name="const", bufs=1) as const, \
             tc.tile_pool(name="persist", bufs=1) as persist:
            # constants
            ident_f = const.tile([P, P], f32)
            make_identity(nc, ident_f)
            ident_r = const.tile([P, P], f32r)
            nc.gpsimd.tensor_copy(out=ident_r[:], in_=ident_f[:])
            ones_sb = const.tile([1, 512], f32r)
            nc.sync.dma_start(ones_sb[:], xT_d[E : E + 1, 0:512])
            b_sb = const.tile([1, 3 * HPC * D], f32r)
            nc.sync.dma_start(b_sb[:], wqkv_d[E : E + 1, :])

            # persistent weights / activations
            w_sb = persist.tile([P, 8, 3 * HPC * D], f32r)
            nc.sync.dma_start(
                w_sb[:], wqkv_d[0:E, :].rearrange("(t p) f -> p t f", p=P)
            )
            wout_sb = persist.tile([P, 2, E], f32r)
            nc.sync.dma_start(
                wout_sb[:], wout_d[:, :].rearrange("(t p) e -> p t e", p=P)
            )
            qkT_sb = persist.tile([P, 4, S], f32r)   # [Q01, Q23, K01, K23]
            v_sb = persist.tile([P, NQT, HPC, P], f32r)
            zero_f = const.tile([P, 512], f32)
            nc.gpsimd.memset(zero_f[:], 0.0)
            for st in range(NQT):
                nc.gpsimd.tensor_copy(
                    out=v_sb[:, st],
                    in_=zero_f[:, :].rearrange("p (a b) -> p a b", a=HPC),
                )
            attT_sb = persist.tile([P, 2, S], f32r)  # [pair, q]

            for rep in range(reps):
                _emit_body(nc, tc, xT_d, attn_d, outp_d,
                           ident_r, ones_sb, b_sb, w_sb, wout_sb,
                           qkT_sb, v_sb, attT_sb)
    nc.compile()
    return nc


def _emit_body(nc, tc, xT_d, attn_d, outp_d, ident_r, ones_sb, b_sb,
               w_sb, wout_sb, qkT_sb, v_sb, attT_sb):
    F = 3 * HPC * D  # 768

    # ---------- Phase 1: QKV projection ----------
    with tc.tile_pool(name="xt", bufs=3) as xt_pool, \
         tc.tile_pool(name="ps1", bufs=4, space="PSUM") as ps1:
        for sc in range(4):  # s-chunks of 512
            qk_ps = [ps1.tile([P, 512], f32, tag="qk", name=f"qk_ps{sc}_{f}") for f in range(4)]
            v_ps = [ps1.tile([P, 256], f32, tag="v", name=f"v_ps{sc}_{st}") for st in range(4)]
            for ke in range(8):
                xt = xt_pool.tile([P, 512], f32r, tag="xt")
                nc.sync.dma_start(
                    xt[:], xT_d[ke * P : (ke + 1) * P, sc * 512 : (sc + 1) * 512]
                )
                for f in range(4):
                    nc.tensor.matmul(
                        qk_ps[f][:],
                        lhsT=w_sb[:, ke, f * P : (f + 1) * P],
                        rhs=xt[:],
                        start=(ke == 0), stop=False,
                    )
                for st in range(4):
                    nc.tensor.matmul(
                        v_ps[st][:],
                        lhsT=xt[:, st * P : (st + 1) * P],
                        rhs=w_sb[:, ke, 512:768],
                        start=(ke == 0), stop=False,
                    )
            for f in range(4):
                nc.tensor.matmul(
                    qk_ps[f][:],
                    lhsT=b_sb[:1, f * P : (f + 1) * P],
                    rhs=ones_sb[:1, :],
                    start=False, stop=True,
                )
                nc.scalar.copy(qkT_sb[:, f, sc * 512 : (sc + 1) * 512], qk_ps[f][:])
            for st in range(4):
                stg = sc * 4 + st
                nc.tensor.matmul(
                    v_ps[st][:],
                    lhsT=ones_sb[:1, 0:P],
                    rhs=b_sb[:1, 512:768],
                    start=False, stop=True,
                )
                for h in range(HPC):
                    po = 64 * (h % 2)
                    nc.vector.tensor_copy(
                        out=v_sb[:, stg, h, po : po + 64],
                        in_=v_ps[st][:, h * 64 : (h + 1) * 64],
                    )

    # ---------- Phase 2: per-head causal attention ----------
    with tc.tile_pool(name="un", bufs=2) as un_pool, \
         tc.tile_pool(name="attnT", bufs=2) as attnT_pool, \
         tc.tile_pool(name="stp", bufs=8) as s_pool, \
         tc.tile_pool(name="ps_sc", bufs=2, space="PSUM") as ps_sc, \
         tc.tile_pool(name="ps_tr", bufs=4, space="PSUM") as ps_tr, \
         tc.tile_pool(name="ps_att", bufs=2, space="PSUM") as ps_att:
        for h in range(HPC):
            po = 64 * (h % 2)
            ftq, ftk, pr = h // 2, 2 + h // 2, h // 2
            for qc in range(S // QC2):  # 8 chunks of 256
                attnT_t = attnT_pool.tile([P, NQT, QC2], f32r, tag="attnT")
                for qsub in range(QC2 // P):
                    i = qc * 2 + qsub
                    nk = i + 1
                    W = nk * P
                    attn_un = un_pool.tile([P, S], f32r, tag="un")
                    stile = s_pool.tile([P, 8], f32, tag="st")
                    scol = 0
                    nch = (W + 511) // 512
                    for kc in range(nch):
                        w = min(512, W - kc * 512)
                        wmm = min(max(w, 256), S - kc * 512)
                        ps = ps_sc.tile([P, wmm], f32, tag="sc")
                        nc.tensor.matmul(
                            ps[:],
                            lhsT=qkT_sb[po : po + 64, ftq, i * P : (i + 1) * P],
                            rhs=qkT_sb[po : po + 64, ftk, kc * 512 : kc * 512 + wmm],
                            start=True, stop=True,
                        )
                        if kc < nch - 1:
                            nc.scalar.activation(
                                attn_un[:, kc * 512 : kc * 512 + w], ps[:, :w],
                                EXP, scale=0.125,
                                accum_out=stile[:, scol : scol + 1],
                            )
                            scol += 1
                        else:
                            wd = w - P
                            if wd > 0:
                                nc.scalar.activation(
                                    attn_un[:, kc * 512 : kc * 512 + wd], ps[:, :wd],
                                    EXP, scale=0.125,
                                    accum_out=stile[:, scol : scol + 1],
                                )
                                scol += 1
                            nc.scalar.activation(
                                attn_un[:, W - P : W], ps[:, wd : wd + P],
                                EXP, scale=0.125,
                            )
                            nc.gpsimd.affine_select(
                                out=attn_un[:, W - P : W],
                                in_=attn_un[:, W - P : W],
                                pattern=[[-1, P]], channel_multiplier=1,
                                compare_op=IS_GE, fill=0.0,
                            )
                            nc.vector.reduce_sum(
                                stile[:, scol : scol + 1],
                                attn_un[:, W - P : W], axis=AX,
                            )
                            scol += 1
                    nc.vector.reduce_sum(stile[:, 6:7], stile[:, 0:scol], axis=AX)
                    nc.vector.reciprocal(stile[:, 7:8], stile[:, 6:7])
                    nc.scalar.mul(attn_un[:, :W], attn_un[:, :W], stile[:, 7:8])
                    nc.sync.dma_start(
                        attn_d[h, i * P : (i + 1) * P, 0:W], attn_un[:, :W]
                    )
                    for kb in range(nk):
                        pt = ps_tr.tile([P, P], f32r, tag="tr")
                        nc.tensor.transpose(
                            pt[:], attn_un[:, kb * P : (kb + 1) * P], ident_r[:]
                        )
                        nc.vector.tensor_copy(
                            out=attnT_t[:, kb, qsub * P : (qsub + 1) * P], in_=pt[:]
                        )
                # attended for this (h, qc)
                pa = ps_att.tile([P, QC2], f32, tag="att")
                last = 2 * qc + 1
                for kb in range(2 * (qc + 1)):
                    off = max(0, (kb - 2 * qc) * P)
                    nc.tensor.matmul(
                        pa[:, off:QC2],
                        lhsT=v_sb[:, kb, h, :],
                        rhs=attnT_t[:, kb, off:QC2],
                        start=(kb == 0), stop=(kb == last),
                    )
                nc.scalar.copy(
                    attT_sb[po : po + 64, pr, qc * QC2 : (qc + 1) * QC2],
                    pa[po : po + 64, :],
                )

    # ---------- Phase 3: output projection ----------
    with tc.tile_pool(name="outt", bufs=2) as out_pool, \
         tc.tile_pool(name="ps_o", bufs=2, space="PSUM") as ps_o:
        for st in range(NQT):
            ot = out_pool.tile([P, E], f32, tag="out")
            for ec in range(2):
                pso = ps_o.tile([P, 512], f32, tag="o")
                for ft in range(2):
                    nc.tensor.matmul(
                        pso[:],
                        lhsT=attT_sb[:, ft, st * P : (st + 1) * P],
                        rhs=wout_sb[:, ft, ec * 512 : (ec + 1) * 512],
                        start=(ft == 0), stop=(ft == 1),
                    )
                nc.vector.tensor_copy(out=ot[:, ec * 512 : (ec + 1) * 512], in_=pso[:])
            nc.sync.dma_start(outp_d[st * P : (st + 1) * P, :], ot[:])


def _get_prog(reps=1):
    if reps not in _prog_cache:
        _prog_cache[reps] = _build(reps)
    return _prog_cache[reps]


def _make_in_maps(x, W_qkv, b_qkv, W_out):
    xT = {}
    for b in range(B):
        xa = np.empty((E + 1, S), dtype=np.float32)
        xa[:E] = x[b].T
        xa[E] = 1.0
        xT[b] = xa
    in_maps = []
    for c in range(NCORES):
        b, g = c // 4, c % 4
        cs = HPC * D * g  # 256*g
        wq = np.empty((E + 1, 3 * HPC * D), dtype=np.float32)
        wq[:E, 0:256] = W_qkv[:, cs : cs + 256]
        wq[:E, 256:512] = W_qkv[:, E + cs : E + cs + 256]
        wq[:E, 512:768] = W_qkv[:, 2 * E + cs : 2 * E + cs + 256]
        wq[E, 0:256] = b_qkv[cs : cs + 256]
        wq[E, 256:512] = b_qkv[E + cs : E + cs + 256]
        wq[E, 512:768] = b_qkv[2 * E + cs : 2 * E + cs + 256]
        in_maps.append({
            "xT": xT[b],
            "wqkv": wq,
            "wout": np.ascontiguousarray(W_out[cs : cs + 256, :], dtype=np.float32),
        })
    return in_maps


def _run(nc, in_maps):
    return run_bass_kernel_spmd(nc, in_maps, core_ids=list(range(NCORES)))


def kernel(x, W_qkv, b_qkv, W_out, b_out):
    x = np.asarray(x, dtype=np.float32)
    W_qkv = np.asarray(W_qkv, dtype=np.float32)
    b_qkv = np.asarray(b_qkv, dtype=np.float32)
    W_out = np.asarray(W_out, dtype=np.float32)
    b_out = np.asarray(b_out, dtype=np.float32)

    nc = _get_prog(1)
    res = _run(nc, _make_in_maps(x, W_qkv, b_qkv, W_out)).results

    attn_w = np.concatenate([res[c]["attn"] for c in range(NCORES)], axis=0)
    attn_w = attn_w.reshape(B, H, S, S)
    out = np.empty((B, S, E), dtype=np.float32)
    for b in range(B):
        acc = res[4 * b]["outp"].copy()
        for c in range(4 * b + 1, 4 * b + 4):
            acc += res[c]["outp"]
        out[b] = acc + b_out
    return out, attn_w
